# revision 12
# baseline (speedup 1.0000x reference)
"""Trainium2 Bass kernel for nn_DDR_coarse (spherical U-Net with GMMConv).

8 NeuronCores, SPMD (one program, per-core index/data inputs):
- ico6/ico5 levels dst-sharded (contiguous vertex ranges, multiples of 128);
  conv outputs exchanged via ncfw AllGather (fp16 rows).
- ico4..ico1 replicated on every core (no exchange).
- GMMConv: fp16 xg-row tables built on PE (per-core-block DMA-transpose of the
  fp16 activation table feeds chunk-stationary matmuls); edge phase = indirect
  row gathers + DVE Gaussian weighting + per-degree-group segment-sum matmuls
  (pow2-padded degrees, fixed 0/1 stationaries) + indirect scatter of per-dst
  means; dense phase = root matmul + bias + leaky-relu in fp32 PSUM.
- Label head: one-ring + softmax rows emitted in exactly the order the first
  label pool consumes them; remaining pools replicated.
Host-side prep is index-only (sorting/grouping/padding + permuting pseudo).
"""
import numpy as np
from contextlib import ExitStack

import concourse.bass as bass
import concourse.tile as tile
from concourse import bacc, mybir
from concourse.ap import AP
from concourse import bass_utils

F32 = mybir.dt.float32
F16 = mybir.dt.float16
I32 = mybir.dt.int32

N_CORES = 8
VER6 = 40962
K = 3
EPS = 1e-15
nf = [32, 64, 128, 256, 512]
OOB = 1 << 27
IN_CH_PAD = 16

# level li = 0..5 <-> ico(6-li). NJ = gather-batch chunk count per level.
LEVELS = [
    dict(N=40962, sharded=True, NJ=16),
    dict(N=10242, sharded=True, NJ=16),
    dict(N=2562, sharded=False, NJ=8),
    dict(N=642, sharded=False, NJ=4),
    dict(N=162, sharded=False, NJ=2),
    dict(N=42, sharded=False, NJ=2),
]
for lv in LEVELS:
    if lv["sharded"]:
        lv["SL"] = ((lv["N"] + N_CORES - 1) // N_CORES + 127) // 128 * 128
        lv["Npad"] = lv["SL"] * N_CORES
    else:
        lv["Npad"] = (lv["N"] + 127) // 128 * 128
        lv["SL"] = lv["Npad"]

CONVS = [
    ("conv1", 0, [IN_CH_PAD], nf[0]), ("conv1_s", 0, [nf[0]], nf[0]),
    ("conv2", 1, [nf[0]], nf[1]), ("conv2_s", 1, [nf[1]], nf[1]),
    ("conv3", 2, [nf[1]], nf[2]), ("conv3_s", 2, [nf[2]], nf[2]),
    ("conv4", 3, [nf[2]], nf[3]), ("conv4_s", 3, [nf[3]], nf[3]),
    ("conv5", 4, [nf[3]], nf[4]), ("conv5_s", 4, [nf[4]], nf[4]),
    ("conv6", 5, [nf[4]], nf[4]), ("conv6_s", 5, [nf[4]], nf[4]),
    ("conv7", 4, [nf[4], nf[3]], nf[3]), ("conv7_s", 4, [nf[3]], nf[3]),
    ("conv8", 3, [nf[3], nf[2]], nf[2]), ("conv8_s", 3, [nf[2]], nf[2]),
    ("conv9", 2, [nf[2], nf[1]], nf[1]), ("conv9_s", 2, [nf[1]], nf[1]),
    ("conv10", 1, [nf[1], nf[0]], nf[0]), ("conv10_s", 1, [nf[0]], nf[0]),
    ("conv11", 0, [nf[0], IN_CH_PAD], nf[0]), ("conv11_s", 0, [nf[0]], nf[0]),
]
CONV_CFG = {c[0]: c for c in CONVS}
DS = [1, 2, 4, 8, 16, 32, 64, 128]


def next_pow2(x):
    return 1 << int(np.ceil(np.log2(max(int(x), 1))))


# ---------------------------------------------------------------------------
# host prep
# ---------------------------------------------------------------------------

def _core_range(li, core):
    lv = LEVELS[li]
    if lv["sharded"]:
        lo = core * lv["SL"]
        hi = min(lo + lv["SL"], lv["N"])
        return lo, hi, lv["SL"]
    return 0, lv["N"], lv["Npad"]


def build_level_plan(src, dst, pseudo, li):
    """SPMD-uniform edge plan for one level: identical chunk layout on every
    core (per-degree chunk counts padded to the max across cores)."""
    lv = LEVELS[li]
    NJ = lv["NJ"]
    percore = []
    for c in range(N_CORES):
        lo, hi, SLloc = _core_range(li, c)
        sel = (dst >= lo) & (dst < hi)
        e_src = src[sel].astype(np.int64)
        e_dst = (dst[sel] - lo).astype(np.int64)
        e_psd = pseudo[sel]
        cnt = np.bincount(e_dst, minlength=SLloc).astype(np.int64)
        order = np.argsort(e_dst, kind="stable")
        e_src, e_psd = e_src[order], e_psd[order]
        estart = np.zeros(SLloc + 1, np.int64)
        np.cumsum(cnt, out=estart[1:])
        assert cnt.max() <= 128, "vertex degree > 128 unsupported"
        dpad = np.zeros(SLloc, np.int64)
        nz = cnt > 0
        dpad[nz] = [next_pow2(c_) for c_ in cnt[nz]]
        percore.append(dict(cnt=cnt, estart=estart, e_src=e_src, e_psd=e_psd,
                            dpad=dpad, SLloc=SLloc))

    # unified per-degree chunk counts
    nch_d = {}
    for d in DS:
        mx = 0
        for pc in percore:
            verts = int((pc["dpad"] == d).sum())
            vpc = 128 // d
            mx = max(mx, (verts + vpc - 1) // vpc)
        if mx:
            nch_d[d] = mx
    total_chunks = sum(nch_d.values())
    nb = max(1, (total_chunks + NJ - 1) // NJ)

    # chunk -> (d, in-group chunk idx) layout, identical on all cores
    layout = []
    for d in DS:
        for j in range(nch_d.get(d, 0)):
            layout.append((d, j))
    while len(layout) < nb * NJ:
        layout.append((None, None))

    # runs (identical across cores): maximal same-d spans within a batch,
    # also capped so n*cout <= 512 at matmul time (cap applied in builder).
    runs = []
    i = 0
    while i < len(layout):
        d, j = layout[i]
        if d is None:
            i += 1
            continue
        b, j0 = i // NJ, i % NJ
        n = 1
        while (i + n < len(layout) and layout[i + n][0] == d
               and (j0 + n) < NJ):
            n += 1
        runs.append((d, b, j0, n, j))  # j = chunk offset in group
        i += n

    # per-core arrays
    arrs = []
    for c, pc in enumerate(percore):
        verts_by_d = {d: np.nonzero(pc["dpad"] == d)[0] for d in nch_d}
        src_t = np.zeros((128, nb * NJ), np.int32)
        psd_t = np.full((128, nb * NJ * 2), 1e6, np.float32)
        scat = {d: np.full((128 // d, nch_d[d]), OOB, np.int32) for d in nch_d}
        icnt = {d: np.zeros((128 // d, nch_d[d]), np.float32) for d in nch_d}
        for ci, (d, j) in enumerate(layout):
            if d is None:
                continue
            vpc = 128 // d
            verts = verts_by_d[d]
            s = np.zeros(128, np.int64)
            p = np.full((128, 2), 1e6, np.float32)
            for q in range(vpc):
                vi = j * vpc + q
                if vi >= len(verts):
                    break
                v = verts[vi]
                e0 = pc["estart"][v]
                ecnt = int(pc["cnt"][v])
                s[q * d:q * d + ecnt] = pc["e_src"][e0:e0 + ecnt]
                p[q * d:q * d + ecnt] = pc["e_psd"][e0:e0 + ecnt]
                scat[d][q, j] = v
                icnt[d][q, j] = 1.0 / max(ecnt, 1)
            src_t[:, ci] = s
            psd_t[:, 2 * ci:2 * ci + 2] = p
        arrs.append(dict(src=src_t, psd=psd_t, scat=scat, icnt=icnt))
    return dict(nb=nb, NJ=NJ, runs=runs, nch_d=nch_d, arrs=arrs)


def prep_inputs(inputs):
    hexes = [np.asarray(h, np.int64) for h in inputs["hexes"]]
    eidx = [np.asarray(e, np.int64) for e in inputs["edge_indexes"]]
    psds = [np.asarray(p, np.float32) for p in inputs["pseudos"]]
    ups = [np.asarray(u, np.int64) for u in inputs["upsamples"]]
    params = inputs["params"]

    meta = {"plans": [], "kparts": {}, "lab": {}}
    per_core = [dict() for _ in range(N_CORES)]

    x_in = np.zeros((LEVELS[0]["Npad"], IN_CH_PAD), np.float32)
    x_in[:VER6, :4] = np.asarray(inputs["moving_img"], np.float32)
    x_in[:VER6, 4:8] = np.asarray(inputs["target_img"], np.float32)

    for li in range(6):
        plan = build_level_plan(eidx[li][0], eidx[li][1], psds[li], li)
        meta["plans"].append(plan)
        for c in range(N_CORES):
            a = plan["arrs"][c]
            per_core[c][f"esrc{li}"] = a["src"]
            per_core[c][f"epsd{li}"] = a["psd"]
            for d in plan["nch_d"]:
                per_core[c][f"scat{li}_d{d}"] = a["scat"][d]
                per_core[c][f"icnt{li}_d{d}"] = a["icnt"][d]

    # dense-phase local row ids
    for li in range(6):
        lv = LEVELS[li]
        for c in range(N_CORES):
            lo = c * lv["SL"] if lv["sharded"] else 0
            SLloc = lv["SL"] if lv["sharded"] else lv["Npad"]
            ids = (lo + np.arange(SLloc)).astype(np.int32)
            per_core[c][f"dgath{li}"] = ids.reshape(SLloc // 128, 128).T.copy()

    # pools (encoder), pool li: level li -> li+1, hexes[li]
    for li in range(5):
        n_out = LEVELS[li + 1]["N"]
        hx = hexes[li][:n_out]
        out_sharded = LEVELS[li + 1]["sharded"]
        for c in range(N_CORES):
            if out_sharded:
                lo = c * LEVELS[li + 1]["SL"]
                hi = min(lo + LEVELS[li + 1]["SL"], n_out)
                SLloc = LEVELS[li + 1]["SL"]
            else:
                lo, hi, SLloc = 0, n_out, LEVELS[li + 1]["Npad"]
            idx = np.zeros((SLloc, 7), np.int64)
            idx[:hi - lo] = hx[lo:hi]
            nch = SLloc // 128
            it = np.zeros((128, nch * 7), np.int32)
            for j in range(nch):
                it[:, 7 * j:7 * j + 7] = idx[j * 128:(j + 1) * 128]
            per_core[c][f"pool{li}_idx"] = it

    # label pools 2..4 (replicated) use hexes[1..3]
    for li in [1, 2, 3]:
        n_out = LEVELS[li + 1]["N"]
        hx = hexes[li][:n_out]
        SLloc = LEVELS[li + 1]["Npad"]
        idx = np.zeros((SLloc, 7), np.int64)
        idx[:n_out] = hx
        nch = SLloc // 128
        it = np.zeros((128, nch * 7), np.int32)
        for j in range(nch):
            it[:, 7 * j:7 * j + 7] = idx[j * 128:(j + 1) * 128]
        for c in range(N_CORES):
            per_core[c][f"plab{li}_idx"] = it

    # hex_up to level li (decoder): pairs
    for li in range(5):
        lv = LEVELS[li]
        Nprev = LEVELS[li + 1]["N"]
        up = ups[li]
        for c in range(N_CORES):
            lo, hi, SLloc = _core_range(li, c)
            lo = c * lv["SL"] if lv["sharded"] else 0
            SLloc = lv["SL"] if lv["sharded"] else lv["Npad"]
            pairs = np.zeros((SLloc, 2), np.int64)
            for j in range(SLloc):
                v = lo + j
                if v < Nprev:
                    pairs[j] = (v, v)
                elif v < lv["N"]:
                    pairs[j] = up[v - Nprev]
            nch = SLloc // 128
            it = np.zeros((128, nch * 2), np.int32)
            for j in range(nch):
                it[:, 2 * j:2 * j + 2] = pairs[j * 128:(j + 1) * 128]
            per_core[c][f"up{li}_idx"] = it

    # label head: per-core lab-row sequence = flatten(hexes[0][range5_c])
    SL5 = LEVELS[1]["SL"]
    n5 = LEVELS[1]["N"]
    hx6 = hexes[0][:n5]
    nrow_pad = SL5 * 7 // 128 * 128
    if SL5 * 7 % 128:
        nrow_pad += 128
    meta["lab"]["nrow_pad"] = nrow_pad
    for c in range(N_CORES):
        lo = c * SL5
        hi = min(lo + SL5, n5)
        seq = np.zeros((nrow_pad,), np.int64)
        seq[:(hi - lo) * 7] = hx6[lo:hi].ravel()
        hxl = np.zeros((nrow_pad, 7), np.int64)
        valid = seq < VER6
        hxl[valid] = hexes[0][seq[valid]]
        nch = nrow_pad // 128
        it = np.zeros((128, nch * 7), np.int32)
        for j in range(nch):
            it[:, 7 * j:7 * j + 7] = hxl[j * 128:(j + 1) * 128]
        per_core[c]["lab_idx"] = it

    # params
    shared = {}

    def put(name, arr):
        shared[name] = np.ascontiguousarray(arr, np.float32)

    for name, li, parts, cout in CONVS:
        p = params[name]
        g = np.asarray(p["g"], np.float32)
        root = np.asarray(p["root"], np.float32)
        cin_pad = sum(parts)
        gp = np.zeros((cin_pad, K * cout), np.float32)
        rp = np.zeros((cin_pad, cout), np.float32)
        if name == "conv1":
            gp[:8] = g
            rp[:8] = root
        elif name == "conv11":
            gp[:nf[0]] = g[:nf[0]]
            gp[nf[0]:nf[0] + 8] = g[nf[0]:]
            rp[:nf[0]] = root[:nf[0]]
            rp[nf[0]:nf[0] + 8] = root[nf[0]:]
        else:
            gp[:g.shape[0]] = g
            rp[:root.shape[0]] = root
        kparts = []
        off = 0
        for ch in parts:
            o2 = 0
            while o2 < ch:
                kp = min(128, ch - o2)
                kparts.append((off + o2, kp))
                o2 += kp
            off += ch
        meta["kparts"][name] = kparts
        for i, (o, kp) in enumerate(kparts):
            put(f"g_{name}_{i}", gp[o:o + kp])
            put(f"root_{name}_{i}", rp[o:o + kp])
        put(f"bias_{name}", np.tile(np.asarray(p["bias"], np.float32)[None, :], (128, 1)))
        mu = np.asarray(p["mu"], np.float32)
        sg = np.asarray(p["sigma"], np.float32)
        put(f"mu_{name}", np.tile(mu.ravel()[None, :], (128, 1)))
        put(f"sg_{name}", np.tile((-0.5 / (sg.ravel() ** 2 + EPS))[None, :], (128, 1)))

    W = np.asarray(params["conv_label"]["W"], np.float32)
    put("labW_0", W[:112])
    put("labW_1", W[112:])
    put("lab_b", np.tile(np.asarray(params["conv_label"]["b"], np.float32)[None, :], (128, 1)))
    for d in DS:
        S = np.zeros((128, 128 // d), np.float32)
        for e in range(128):
            S[e, e // d] = 1.0
        put(f"S_d{d}", S)

    for c in range(N_CORES):
        per_core[c]["x_in"] = x_in
        per_core[c].update(shared)
    return meta, per_core


# ---------------------------------------------------------------------------
# bass program
# ---------------------------------------------------------------------------

XT_BLK = {0: 5248, 1: 1408, 2: 2688, 3: 768, 4: 256, 5: 128}


class Builder:
    def __init__(self, nc, tc, ctx, meta):
        self.nc, self.tc, self.meta = nc, tc, meta
        self.dram = ctx.enter_context(tc.tile_pool(name="dram", bufs=1, space="DRAM"))
        self.sb = ctx.enter_context(tc.tile_pool(name="sb", bufs=1))
        self.wk = ctx.enter_context(tc.tile_pool(name="wk", bufs=2))
        self.ps = ctx.enter_context(tc.tile_pool(name="ps", bufs=2, space="PSUM"))
        self.ins = {}
        self.uid = 0

    def nid(self, pfx):
        self.uid += 1
        return f"{pfx}{self.uid}"

    def inp(self, name, shape, dt):
        if name not in self.ins:
            self.ins[name] = self.nc.dram_tensor(name, list(shape), dt, kind="ExternalInput")
        return self.ins[name]

    def load_const(self, name, shape, tag, cast16=False):
        t = self.inp(name, shape, F32)
        if cast16:
            tl16 = self.sb.tile(list(shape), F16, name=self.nid("k"), tag=tag + "h")
            self.nc.gpsimd.dma_start(out=tl16[:], in_=t.ap())
            return tl16
        tl = self.sb.tile(list(shape), F32, name=self.nid("k"), tag=tag)
        self.nc.sync.dma_start(out=tl[:], in_=t.ap())
        return tl

    def conv(self, name, in_tables):
        nc = self.nc
        _, li, parts, cout = CONV_CFG[name]
        lv = LEVELS[li]
        Npad, SL, sharded = lv["Npad"], lv["SL"], lv["sharded"]
        SLloc = SL if sharded else Npad
        kparts = self.meta["kparts"][name]
        c3 = K * cout
        plan = self.meta["plans"][li]
        NJ, nb = plan["NJ"], plan["nb"]

        g_tiles = [self.load_const(f"g_{name}_{i}", (kp, c3), f"g{i}", cast16=True)
                   for i, (_, kp) in enumerate(kparts)]
        r_tiles = [self.load_const(f"root_{name}_{i}", (kp, cout), f"r{i}", cast16=True)
                   for i, (_, kp) in enumerate(kparts)]
        bias_t = self.load_const(f"bias_{name}", (128, cout), "bias")
        mu_t = self.load_const(f"mu_{name}", (128, 6), "mu")
        sg_t = self.load_const(f"sg_{name}", (128, 6), "sg")

        # ---- xg table build (per-chunk PE transpose + matmul) ----
        xg_tab = self.dram.tile([Npad, c3], F16, name=f"xg_{name}")
        n_nt = (c3 + 511) // 512
        nsz = (c3 + n_nt - 1) // n_nt
        cin_tot0 = sum(ch for _, ch in in_tables)
        for j in range(Npad // 128):
            xrow = self.wk.tile([128, cin_tot0], F16, tag="xrow", bufs=2)
            o = 0
            for tab, ch in in_tables:
                nc.sync.dma_start(out=xrow[:, o:o + ch],
                                  in_=tab[j * 128:(j + 1) * 128, :])
                o += ch
            xts_l = []
            for i, (off, kp) in enumerate(kparts):
                xtp = self.ps.tile([128, 128], F16, tag="xtps", space="PSUM")
                nc.tensor.transpose(out=xtp[:kp, :], in_=xrow[:, off:off + kp],
                                    identity=self.ident16[:])
                xts = self.wk.tile([128, 128], F16, tag=f"bxts{i}")
                nc.vector.tensor_copy(out=xts[:kp, :], in_=xtp[:kp, :])
                xts_l.append(xts)
            wide = self.wk.tile([128, c3], F16, tag="xgw", bufs=2)
            for nt in range(n_nt):
                n0 = nt * nsz
                n1 = min(n0 + nsz, c3)
                psb = self.ps.tile([128, 512], F32, tag="mmps", space="PSUM")
                for i, (off, kp) in enumerate(kparts):
                    nc.tensor.matmul(out=psb[:, :n1 - n0],
                                     lhsT=xts_l[i][:kp, :],
                                     rhs=g_tiles[i][:, n0:n1],
                                     start=(i == 0), stop=(i == len(kparts) - 1))
                nc.vector.tensor_copy(out=wide[:, n0:n1], in_=psb[:, :n1 - n0])
            dst = xg_tab[:]
            dap = AP(dst.tensor, dst.offset + j * 128 * c3, [[c3, 128], [1, c3]])
            nc.sync.dma_start(out=dap, in_=wide[:])

        # ---- edge phase ----
        esrc = self.inp(f"esrc{li}", (128, nb * NJ), I32)
        epsd = self.inp(f"epsd{li}", (128, nb * NJ * 2), F32)
        esrc_t = self.sb.tile([128, nb * NJ], I32, name=self.nid("es"), tag="esrc")
        nc.sync.dma_start(out=esrc_t[:], in_=esrc.ap())
        epsd_t = self.sb.tile([128, nb * NJ * 2], F32, name=self.nid("ep"), tag="epsd")
        nc.sync.dma_start(out=epsd_t[:], in_=epsd.ap())

        s_tab = self.dram.tile([SLloc, cout], F16, name=f"s_{name}")
        zt = self.wk.tile([128, (SLloc // 128) * cout], F16, tag="zt", bufs=1)
        nc.vector.memset(zt[:], 0.0)
        nc.sync.dma_start(out=s_tab[:].rearrange("(p a) c -> p (a c)", p=128), in_=zt[:])

        gout, gscat, gicnt = {}, {}, {}
        for d, nch in plan["nch_d"].items():
            gout[d] = self.sb.tile([128 // d, nch * cout], F16,
                                   name=self.nid("go"), tag=f"go{d}")
            st = self.inp(f"scat{li}_d{d}", (128 // d, nch), I32)
            stt = self.sb.tile([128 // d, nch], I32, name=self.nid("sc"), tag=f"sc{d}")
            nc.sync.dma_start(out=stt[:], in_=st.ap())
            gscat[d] = stt
            ic = self.inp(f"icnt{li}_d{d}", (128 // d, nch), F32)
            ict = self.sb.tile([128 // d, nch], F32, name=self.nid("ic"), tag=f"ic{d}")
            nc.sync.dma_start(out=ict[:], in_=ic.ap())
            gicnt[d] = ict

        runs_by_batch = {}
        n_max = max(1, 512 // cout)
        for (d, b, j0, n, coff) in plan["runs"]:
            o = 0
            while o < n:
                nn = min(n_max, n - o)
                runs_by_batch.setdefault(b, []).append((d, j0 + o, nn, coff + o))
                o += nn

        for b in range(nb):
            gt = self.wk.tile([128, NJ * c3], F16, tag="gt")
            for jj in range(NJ):
                nc.gpsimd.indirect_dma_start(
                    out=gt[:, jj * c3:(jj + 1) * c3], out_offset=None, in_=xg_tab[:],
                    in_offset=bass.IndirectOffsetOnAxis(
                        ap=esrc_t[:, b * NJ + jj:b * NJ + jj + 1], axis=0))
            pap = epsd_t[:, :]
            psd6 = AP(pap.tensor, pap.offset + b * NJ * 2,
                      [list(pap.ap[0]), [2, NJ], [0, 3], [1, 2]])
            dt_ = self.wk.tile([128, NJ * 6], F32, tag="d6")
            map_ = mu_t[:, :]
            mu_b = AP(map_.tensor, map_.offset, [list(map_.ap[0]), [0, NJ], [1, 6]])
            nc.vector.tensor_tensor(out=dt_[:], in0=psd6, in1=mu_b,
                                    op=mybir.AluOpType.subtract)
            nc.vector.tensor_tensor(out=dt_[:], in0=dt_[:], in1=dt_[:],
                                    op=mybir.AluOpType.mult)
            sap_ = sg_t[:, :]
            sg_b = AP(sap_.tensor, sap_.offset, [list(sap_.ap[0]), [0, NJ], [1, 6]])
            nc.vector.tensor_tensor(out=dt_[:], in0=dt_[:], in1=sg_b,
                                    op=mybir.AluOpType.mult)
            wt = self.wk.tile([128, NJ * 3], F32, tag="wt")
            dap_ = dt_[:, :]
            s0 = AP(dap_.tensor, dap_.offset, [list(dap_.ap[0]), [6, NJ], [2, 3]])
            s1 = AP(dap_.tensor, dap_.offset + 1, [list(dap_.ap[0]), [6, NJ], [2, 3]])
            nc.vector.tensor_tensor(out=wt[:], in0=s0, in1=s1, op=mybir.AluOpType.add)
            wt16 = self.wk.tile([128, NJ * 3], F32, tag="wt16")
            nc.scalar.activation(wt16[:], wt[:], mybir.ActivationFunctionType.Exp)
            t1 = self.wk.tile([128, NJ * c3], F32, tag="t1")
            wap = wt16[:, :]
            w_b = AP(wap.tensor, wap.offset, [list(wap.ap[0]), [3, NJ], [1, 3], [0, cout]])
            nc.vector.tensor_tensor(out=t1[:], in0=gt[:], in1=w_b, op=mybir.AluOpType.mult)
            msg = self.wk.tile([128, NJ * cout], F32, tag="msg")
            tap = t1[:, :]

            def sk(k):
                return AP(tap.tensor, tap.offset + k * cout,
                          [list(tap.ap[0]), [c3, NJ], [1, cout]])
            nc.vector.tensor_tensor(out=msg[:], in0=sk(0), in1=sk(1), op=mybir.AluOpType.add)
            nc.vector.tensor_tensor(out=msg[:], in0=msg[:], in1=sk(2), op=mybir.AluOpType.add)
            for (d, j0, n, coff) in runs_by_batch.get(b, []):
                pse = self.ps.tile([128, 512], F32, tag="segps", space="PSUM")
                nc.tensor.matmul(out=pse[:128 // d, :n * cout], lhsT=self.S16[d][:],
                                 rhs=msg[:, j0 * cout:(j0 + n) * cout],
                                 start=True, stop=True)
                icap = gicnt[d][:, :]
                ic_b = AP(icap.tensor, icap.offset + coff,
                          [list(icap.ap[0]), [1, n], [0, cout]])
                nc.vector.tensor_tensor(
                    out=gout[d][:, coff * cout:(coff + n) * cout],
                    in0=pse[:128 // d, :n * cout], in1=ic_b, op=mybir.AluOpType.mult)

        for d in gout:
            nch = plan["nch_d"][d]
            for jj in range(nch):
                nc.gpsimd.indirect_dma_start(
                    out=s_tab[:], in_=gout[d][:, jj * cout:(jj + 1) * cout],
                    out_offset=bass.IndirectOffsetOnAxis(ap=gscat[d][:, jj:jj + 1], axis=0),
                    in_offset=None, bounds_check=SLloc - 1, oob_is_err=False)

        # ---- dense phase ----
        out_rows = self.dram.tile([SLloc, cout], F16, name=f"ob_{name}")
        nch_loc = SLloc // 128
        dg = self.inp(f"dgath{li}", (128, nch_loc), I32)
        dg_t = self.sb.tile([128, nch_loc], I32, name=self.nid("dg"), tag="dg")
        nc.sync.dma_start(out=dg_t[:], in_=dg.ap())
        cin_tot = sum(ch for _, ch in in_tables)
        xr_all = self.sb.tile([128, nch_loc * cin_tot], F16,
                              name=self.nid("xr"), tag="xr")
        for j in range(nch_loc):
            o = 0
            for tab, ch in in_tables:
                nc.gpsimd.indirect_dma_start(
                    out=xr_all[:, j * cin_tot + o:j * cin_tot + o + ch],
                    out_offset=None, in_=tab,
                    in_offset=bass.IndirectOffsetOnAxis(ap=dg_t[:, j:j + 1], axis=0))
                o += ch
        for j in range(nch_loc):
            psr = self.ps.tile([128, 512], F32, tag="mmps", space="PSUM")
            for i, (off, kp) in enumerate(kparts):
                xtp = self.ps.tile([128, 128], F16, tag="xtps", space="PSUM")
                nc.tensor.transpose(out=xtp[:kp, :],
                                    in_=xr_all[:, j * cin_tot + off:j * cin_tot + off + kp],
                                    identity=self.ident16[:])
                xts = self.wk.tile([128, 128], F16, tag="xts")
                nc.vector.tensor_copy(out=xts[:kp, :], in_=xtp[:kp, :])
                nc.tensor.matmul(out=psr[:, :cout], lhsT=xts[:kp, :], rhs=r_tiles[i][:],
                                 start=(i == 0), stop=(i == len(kparts) - 1))
            srow = self.wk.tile([128, cout], F16, tag="srow")
            nc.sync.dma_start(out=srow[:], in_=s_tab[:][j * 128:(j + 1) * 128, :])
            comb = self.wk.tile([128, cout], F32, tag="comb")
            nc.vector.tensor_tensor(out=comb[:], in0=srow[:], in1=psr[:, :cout],
                                    op=mybir.AluOpType.add)
            nc.vector.tensor_tensor(out=comb[:], in0=comb[:], in1=bias_t[:, :cout],
                                    op=mybir.AluOpType.add)
            c2 = self.wk.tile([128, cout], F32, tag="c2")
            nc.vector.tensor_scalar(out=c2[:], in0=comb[:], scalar1=0.2, scalar2=None,
                                    op0=mybir.AluOpType.mult)
            o16 = self.wk.tile([128, cout], F16, tag="o16")
            nc.vector.tensor_tensor(out=o16[:], in0=comb[:], in1=c2[:],
                                    op=mybir.AluOpType.max)
            nc.sync.dma_start(out=out_rows[:][j * 128:(j + 1) * 128, :], in_=o16[:])

        if sharded:
            full = self.dram.tile([Npad, cout], F16, name=f"xtab_{name}", addr_space="Shared")
            nc.gpsimd.collective_compute(
                "AllGather", mybir.AluOpType.bypass,
                replica_groups=[list(range(N_CORES))],
                ins=[out_rows.opt()], outs=[full.opt()])
            return full
        return out_rows

    def _mean_gather(self, in_table, chin, idx_tile, nch, fan, out_rows, gb=4):
        """out rows[j*128+p] = mean of `fan` gathered rows; gathers batched."""
        nc = self.nc
        inv = 1.0 / fan
        gb = min(gb, nch)
        for j0 in range(0, nch, gb):
            jn = min(gb, nch - j0)
            g = self.wk.tile([128, gb * fan * chin], F16, tag=f"mg{fan}", bufs=1)
            for q in range(jn * fan):
                nc.gpsimd.indirect_dma_start(
                    out=g[:, q * chin:(q + 1) * chin], out_offset=None, in_=in_table,
                    in_offset=bass.IndirectOffsetOnAxis(
                        ap=idx_tile[:, j0 * fan + q:j0 * fan + q + 1], axis=0))
            for j in range(j0, j0 + jn):
                gap = g[:, :]
                base = (j - j0) * fan * chin

                def sl(t):
                    return AP(gap.tensor, gap.offset + base + t,
                              [list(gap.ap[0]), [fan, chin]])
                acc = self.wk.tile([128, chin], F32, tag="mga")
                nc.vector.tensor_tensor(out=acc[:], in0=sl(0), in1=sl(1),
                                        op=mybir.AluOpType.add)
                for t in range(2, fan):
                    nc.vector.tensor_tensor(out=acc[:], in0=acc[:], in1=sl(t),
                                            op=mybir.AluOpType.add)
                o = self.wk.tile([128, chin], F16, tag="mgo")
                nc.vector.tensor_scalar(out=o[:], in0=acc[:], scalar1=inv, scalar2=None,
                                        op0=mybir.AluOpType.mult)
                nc.sync.dma_start(out=out_rows[:][j * 128:(j + 1) * 128, :], in_=o[:])

    def pool(self, li, in_table, chin, idx_name):
        nc = self.nc
        lv_out = LEVELS[li + 1]
        out_sharded = lv_out["sharded"]
        SLloc = lv_out["SL"] if out_sharded else lv_out["Npad"]
        nch = SLloc // 128
        it = self.inp(idx_name, (128, nch * 7), I32)
        itt = self.sb.tile([128, nch * 7], I32, name=self.nid("pi"), tag="pidx")
        nc.sync.dma_start(out=itt[:], in_=it.ap())
        out_rows = self.dram.tile([SLloc, chin], F16, name=self.nid("pob"))
        self._mean_gather(in_table, chin, itt, nch, 7, out_rows,
                          gb=max(1, 4096 // (7 * chin * 2)))
        if out_sharded:
            full = self.dram.tile([lv_out["Npad"], chin], F16, name=self.nid("pfull"), addr_space="Shared")
            nc.gpsimd.collective_compute(
                "AllGather", mybir.AluOpType.bypass,
                replica_groups=[list(range(N_CORES))],
                ins=[out_rows.opt()], outs=[full.opt()])
            return full
        return out_rows

    def hexup(self, li, in_table, chin):
        nc = self.nc
        lv = LEVELS[li]
        sharded = lv["sharded"]
        SLloc = lv["SL"] if sharded else lv["Npad"]
        nch = SLloc // 128
        it = self.inp(f"up{li}_idx", (128, nch * 2), I32)
        itt = self.sb.tile([128, nch * 2], I32, name=self.nid("ui"), tag="uidx")
        nc.sync.dma_start(out=itt[:], in_=it.ap())
        out_rows = self.dram.tile([SLloc, chin], F16, name=self.nid("uob"))
        self._mean_gather(in_table, chin, itt, nch, 2, out_rows,
                          gb=max(1, 4096 // (2 * chin * 2)))
        if sharded:
            full = self.dram.tile([lv["Npad"], chin], F16, name=self.nid("ufull"), addr_space="Shared")
            nc.gpsimd.collective_compute(
                "AllGather", mybir.AluOpType.bypass,
                replica_groups=[list(range(N_CORES))],
                ins=[out_rows.opt()], outs=[full.opt()])
            return full
        return out_rows

    def label_head(self, x_table):
        nc = self.nc
        nrow_pad = self.meta["lab"]["nrow_pad"]
        nch = nrow_pad // 128
        it = self.inp("lab_idx", (128, nch * 7), I32)
        itt = self.sb.tile([128, nch * 7], I32, name=self.nid("li"), tag="lidx")
        nc.sync.dma_start(out=itt[:], in_=it.ap())
        W0 = self.load_const("labW_0", (112, 162), "W0", cast16=True)
        W1 = self.load_const("labW_1", (112, 162), "W1", cast16=True)
        lb = self.load_const("lab_b", (128, 162), "lb")
        lab_seq = self.dram.tile([nrow_pad, 162], F16, name="lab_seq")
        GB = 8
        for j0 in range(0, nch, GB):
            jn = min(GB, nch - j0)
            g = self.wk.tile([128, GB * 224], F16, tag="labg", bufs=1)
            for q in range(jn * 7):
                nc.gpsimd.indirect_dma_start(
                    out=g[:, q * 32:(q + 1) * 32], out_offset=None, in_=x_table,
                    in_offset=bass.IndirectOffsetOnAxis(
                        ap=itt[:, j0 * 7 + q:j0 * 7 + q + 1], axis=0))
            for j in range(j0, j0 + jn):
                base = (j - j0) * 224
                psl = self.ps.tile([128, 162], F32, tag="mmps", space="PSUM")
                for i, W in enumerate([W0, W1]):
                    xtp = self.ps.tile([128, 128], F16, tag="xtps", space="PSUM")
                    nc.tensor.transpose(out=xtp[:112, :],
                                        in_=g[:, base + i * 112:base + (i + 1) * 112],
                                        identity=self.ident16[:])
                    xts = self.wk.tile([128, 128], F16, tag="xts")
                    nc.vector.tensor_copy(out=xts[:112, :], in_=xtp[:112, :])
                    nc.tensor.matmul(out=psl[:], lhsT=xts[:112, :], rhs=W[:],
                                     start=(i == 0), stop=(i == 1))
                logits = self.wk.tile([128, 162], F32, tag="logits")
                nc.vector.tensor_tensor(out=logits[:], in0=psl[:], in1=lb[:],
                                        op=mybir.AluOpType.add)
                mx = self.wk.tile([128, 1], F32, tag="mx")
                nc.vector.tensor_reduce(out=mx[:], in_=logits[:],
                                        axis=mybir.AxisListType.X, op=mybir.AluOpType.max)
                nmax = self.wk.tile([128, 1], F32, tag="nmax")
                nc.vector.tensor_scalar(out=nmax[:], in0=mx[:], scalar1=-1.0,
                                        scalar2=None, op0=mybir.AluOpType.mult)
                ex = self.wk.tile([128, 162], F32, tag="ex")
                sums = self.wk.tile([128, 1], F32, tag="sums")
                nc.scalar.activation(ex[:], logits[:], mybir.ActivationFunctionType.Exp,
                                     bias=nmax[:, :1], accum_out=sums[:, :1])
                rec = self.wk.tile([128, 1], F32, tag="rec")
                nc.vector.reciprocal(rec[:], sums[:])
                prob = self.wk.tile([128, 162], F16, tag="prob")
                nc.vector.tensor_scalar(out=prob[:], in0=ex[:], scalar1=rec[:, :1],
                                        scalar2=None, op0=mybir.AluOpType.mult)
                nc.sync.dma_start(out=lab_seq[:][j * 128:(j + 1) * 128, :], in_=prob[:])

        # pool-lab1: consume lab_seq contiguously
        SL5 = LEVELS[1]["SL"]
        nch5 = SL5 // 128
        p1 = self.dram.tile([SL5, 162], F16, name="plab1_rows")
        for j in range(nch5):
            flat = self.wk.tile([128, 7 * 162], F16, tag="pl1f")
            src = lab_seq[:]
            sap = AP(src.tensor, src.offset + j * 128 * 7 * 162,
                     [[7 * 162, 128], [1, 7 * 162]])
            nc.sync.dma_start(out=flat[:], in_=sap)
            gap = flat[:, :]

            def sl(t):
                return AP(gap.tensor, gap.offset + t, [list(gap.ap[0]), [7, 162]])
            acc = self.wk.tile([128, 162], F32, tag="mga")
            nc.vector.tensor_tensor(out=acc[:], in0=sl(0), in1=sl(1), op=mybir.AluOpType.add)
            for t in range(2, 7):
                nc.vector.tensor_tensor(out=acc[:], in0=acc[:], in1=sl(t),
                                        op=mybir.AluOpType.add)
            o = self.wk.tile([128, 162], F16, tag="mgo")
            nc.vector.tensor_scalar(out=o[:], in0=acc[:], scalar1=1.0 / 7.0,
                                    scalar2=None, op0=mybir.AluOpType.mult)
            nc.sync.dma_start(out=p1[:][j * 128:(j + 1) * 128, :], in_=o[:])
        p1full = self.dram.tile([LEVELS[1]["Npad"], 162], F16, name="plab1_full", addr_space="Shared")
        nc.gpsimd.collective_compute(
            "AllGather", mybir.AluOpType.bypass,
            replica_groups=[list(range(N_CORES))],
            ins=[p1.opt()], outs=[p1full.opt()])

        t = p1full
        for li in [1, 2, 3]:
            SLloc = LEVELS[li + 1]["Npad"]
            nchl = SLloc // 128
            itl = self.inp(f"plab{li}_idx", (128, nchl * 7), I32)
            itt2 = self.sb.tile([128, nchl * 7], I32, name=self.nid("pli"), tag="pidx")
            nc.sync.dma_start(out=itt2[:], in_=itl.ap())
            ot = self.dram.tile([SLloc, 162], F16, name=self.nid("plob"))
            self._mean_gather(t[:], 162, itt2, nchl, 7, ot, gb=4)
            t = ot

        out = self.nc.dram_tensor("out", [162, 162], F32, kind="ExternalOutput")
        f16t = self.wk.tile([128, 2 * 162], F16, tag="f16t")
        nc.sync.dma_start(out=f16t[:, :162], in_=t[:][0:128, :])
        nc.sync.dma_start(out=f16t[:34, 162:], in_=t[:][128:162, :])
        fin = self.wk.tile([128, 2 * 162], F32, tag="fin")
        nc.vector.tensor_copy(out=fin[:], in_=f16t[:])
        nc.sync.dma_start(out=out.ap()[0:128, :], in_=fin[:, :162])
        nc.sync.dma_start(out=out.ap()[128:162, :], in_=fin[:34, 162:])


class _StageDone(Exception):
    def __init__(self, ins):
        self.ins = ins


def build_program(meta):
    nc = bacc.Bacc("TRN2", target_bir_lowering=False, debug=False,
                   num_devices=N_CORES)
    with tile.TileContext(nc) as tc:
        with ExitStack() as ctx:
          try:
            b = Builder(nc, tc, ctx, meta)
            from concourse.masks import make_identity
            b.ident16 = b.sb.tile([128, 128], F16, name="ident16", tag="ident")
            make_identity(nc, b.ident16[:])
            b.S16 = {d: b.load_const(f"S_d{d}", (128, 128 // d), f"S{d}", cast16=False)
                     for d in DS}

            Np0 = LEVELS[0]["Npad"]
            xin = b.inp("x_in", (Np0, IN_CH_PAD), F32)
            xin16 = b.dram.tile([Np0, IN_CH_PAD], F16, name="xin16")
            CH = 32
            for j0 in range(0, Np0 // 128, CH):
                jn = min(CH, Np0 // 128 - j0)
                t = b.wk.tile([128, CH * IN_CH_PAD], F32, tag="xc")
                src = xin.ap()
                sap = AP(src.tensor, src.offset + j0 * 128 * IN_CH_PAD,
                         [[IN_CH_PAD, 128], [128 * IN_CH_PAD, jn], [1, IN_CH_PAD]])
                nc.sync.dma_start(out=t[:, :jn * IN_CH_PAD], in_=sap)
                t16 = b.wk.tile([128, CH * IN_CH_PAD], F16, tag="xc16")
                nc.vector.tensor_copy(out=t16[:, :jn * IN_CH_PAD], in_=t[:, :jn * IN_CH_PAD])
                dst = xin16[:]
                dap = AP(dst.tensor, dst.offset + j0 * 128 * IN_CH_PAD,
                         [[IN_CH_PAD, 128], [128 * IN_CH_PAD, jn], [1, IN_CH_PAD]])
                nc.sync.dma_start(out=dap, in_=t16[:, :jn * IN_CH_PAD])

            import os as _os
            STAGE = int(_os.environ.get("KERNEL_STAGE", "0"))

            def _dump(tab, rows, ch):
                dbg = nc.dram_tensor("dbg", [rows, ch], F16, kind="ExternalOutput")
                for jj in range(rows // 128):
                    tt = b.wk.tile([128, ch], F16, tag="dbg")
                    nc.sync.dma_start(out=tt[:], in_=tab[jj * 128:(jj + 1) * 128, :])
                    nc.sync.dma_start(out=dbg.ap()[jj * 128:(jj + 1) * 128, :], in_=tt[:])
                out = nc.dram_tensor("out", [162, 162], F32, kind="ExternalOutput")
                zz = b.wk.tile([128, 162], F32, tag="zz")
                nc.vector.memset(zz[:], 0.0)
                nc.sync.dma_start(out=out.ap()[0:128, :], in_=zz[:])
                nc.sync.dma_start(out=out.ap()[128:162, :], in_=zz[:34, :])

            xt_in = xin16[:]
            x1 = b.conv("conv1", [(xt_in, IN_CH_PAD)])
            if STAGE == 1:
                _dump(x1[:], LEVELS[0]["Npad"], nf[0])
                ins = dict(b.ins)
                raise _StageDone(ins)
            x1s = b.conv("conv1_s", [(x1[:], nf[0])])
            p1 = b.pool(0, x1s[:], nf[0], "pool0_idx")
            x2 = b.conv("conv2", [(p1[:], nf[0])])
            x2s = b.conv("conv2_s", [(x2[:], nf[1])])
            if STAGE == 2:
                _dump(x2s[:], LEVELS[1]["Npad"], nf[1])
                ins = dict(b.ins)
                raise _StageDone(ins)
            p2 = b.pool(1, x2s[:], nf[1], "pool1_idx")
            x3 = b.conv("conv3", [(p2[:], nf[1])])
            x3s = b.conv("conv3_s", [(x3[:], nf[2])])
            p3 = b.pool(2, x3s[:], nf[2], "pool2_idx")
            x4 = b.conv("conv4", [(p3[:], nf[2])])
            x4s = b.conv("conv4_s", [(x4[:], nf[3])])
            p4 = b.pool(3, x4s[:], nf[3], "pool3_idx")
            x5 = b.conv("conv5", [(p4[:], nf[3])])
            x5s = b.conv("conv5_s", [(x5[:], nf[4])])
            p5 = b.pool(4, x5s[:], nf[4], "pool4_idx")
            x6 = b.conv("conv6", [(p5[:], nf[4])])
            x6s = b.conv("conv6_s", [(x6[:], nf[4])])
            if STAGE == 3:
                _dump(x6s[:], LEVELS[5]["Npad"], nf[4])
                ins = dict(b.ins)
                raise _StageDone(ins)
            u4 = b.hexup(4, x6s[:], nf[4])
            x7 = b.conv("conv7", [(u4[:], nf[4]), (p4[:], nf[3])])
            x7s = b.conv("conv7_s", [(x7[:], nf[3])])
            u3 = b.hexup(3, x7s[:], nf[3])
            x8 = b.conv("conv8", [(u3[:], nf[3]), (p3[:], nf[2])])
            x8s = b.conv("conv8_s", [(x8[:], nf[2])])
            u2 = b.hexup(2, x8s[:], nf[2])
            x9 = b.conv("conv9", [(u2[:], nf[2]), (p2[:], nf[1])])
            x9s = b.conv("conv9_s", [(x9[:], nf[1])])
            u1 = b.hexup(1, x9s[:], nf[1])
            x10 = b.conv("conv10", [(u1[:], nf[1]), (p1[:], nf[0])])
            x10s = b.conv("conv10_s", [(x10[:], nf[0])])
            u0 = b.hexup(0, x10s[:], nf[0])
            x11 = b.conv("conv11", [(u0[:], nf[0]), (xt_in, IN_CH_PAD)])
            x11s = b.conv("conv11_s", [(x11[:], nf[0])])
            if STAGE == 4:
                _dump(x11s[:], LEVELS[0]["Npad"], nf[0])
                ins = dict(b.ins)
                raise _StageDone(ins)
            b.label_head(x11s[:])
            ins = dict(b.ins)
          except _StageDone as e:
            ins = e.ins
    nc.compile()
    return nc, ins


_CACHE = {}


def kernel(**inputs) -> np.ndarray:
    meta, per_core = prep_inputs(inputs)
    key = tuple((p["nb"], p["NJ"], tuple(sorted(p["nch_d"].items())), tuple(p["runs"]))
                for p in meta["plans"])
    if _CACHE.get("key") != key:
        _CACHE["nc"], _CACHE["ins"] = build_program(meta)
        _CACHE["key"] = key
    nc = _CACHE["nc"]
    in_maps = []
    for c in range(N_CORES):
        m = {}
        for name, t in _CACHE["ins"].items():
            a = np.ascontiguousarray(per_core[c][name])
            assert tuple(a.shape) == tuple(t.shape), (name, a.shape, tuple(t.shape))
            m[name] = a
        in_maps.append(m)
    res = bass_utils.run_bass_kernel_spmd(nc, in_maps, core_ids=list(range(N_CORES)))
    _CACHE["res"] = res
    return np.asarray(res.results[0]["out"], np.float32)
